# revision 2
# baseline (speedup 1.0000x reference)
"""CRF NLL loss kernel for Trainium2 (8 NeuronCores, batch-sharded).

Strategy
--------
Data-parallel over batch: each of 8 cores handles 64 sequences.

Forward algorithm (log-partition) runs in the EXP DOMAIN with labels on
partitions and batch on the free dim:  w_t[l, b] ~= exp(fv_t[l, b] - shift).
One step is a single PE matmul with the stationary weight
E' = exp(transitions - C0) plus one DVE multiply by exp(features_t):

    w_t = exp(feat_t) * (E'^T @ w_{t-1})

No per-step logsumexp / max / mask select.  Variable sequence lengths are
handled by CAPTURE: z_t[b] = exp(trans[:,EOS])^T . w_t[:, b] is computed for
every step (batched, one [1, 512] matmul per 8 steps over a 16-slot ring
buffer) and the value at t = len(b)-1 is selected with host-precomputed 0/1
indicator rows.  Every 16 steps columns are rescaled by 1/colsum (logged via
the reciprocals, un-done on the host in log space).  exp underflow of the
constrained PAD/BOS rows (value -10000) is exact (-> 0.0), matching the
reference's logsumexp to f32 accuracy.

Gold path score: host gathers the indexed scalars feat[b,t,tag] and
trans[tag,tag'] (pure index marshalling, no arithmetic); the device does the
masked weighted sums.

Host post-processing is O(B) logs: logZ = log(C) + t*.C0 + sum(log colsums).
"""

import numpy as np

B, T, L = 512, 512, 128
NCORES = 8
BC = B // NCORES            # 64 sequences per core
PAD, BOS, EOS = 0, 1, 2
C0 = 5.0                    # constant per-step log-shift folded into E'
CH = 8                      # steps per chunk
NCHUNK = T // CH            # 64 chunks (chunk 0 holds init + steps 1..7)
RING = 16                   # w ring slots
NEV = 31                    # rescale events: after t = 15, 31, ..., 495

F32 = np.float32

_compiled = None


def _build():
    import concourse.bass as bass
    import concourse.bacc as bacc
    import concourse.mybir as mybir
    import concourse.tile as tile

    f32 = mybir.dt.float32
    nc = bacc.Bacc("TRN2", target_bir_lowering=False, debug=False)

    featc = nc.dram_tensor("featc", [NCHUNK, L, CH * BC], f32, kind="ExternalInput")
    trans = nc.dram_tensor("trans", [L, L], f32, kind="ExternalInput")
    ind = nc.dram_tensor("ind", [NCHUNK, CH * BC], f32, kind="ExternalInput")
    emis_v = nc.dram_tensor("emis_v", [BC, T], f32, kind="ExternalInput")
    emis_w = nc.dram_tensor("emis_w", [BC, T], f32, kind="ExternalInput")
    trans_v = nc.dram_tensor("trans_v", [BC, T + 1], f32, kind="ExternalInput")
    trans_w = nc.dram_tensor("trans_w", [BC, T + 1], f32, kind="ExternalInput")

    cacc_o = nc.dram_tensor("cacc", [1, CH * BC], f32, kind="ExternalOutput")
    recips_o = nc.dram_tensor("recips", [1, NEV * BC], f32, kind="ExternalOutput")
    gold_o = nc.dram_tensor("gold", [BC, 1], f32, kind="ExternalOutput")

    AX = mybir.AxisListType.X
    MUL = mybir.AluOpType.mult
    ADD = mybir.AluOpType.add
    EXP = mybir.ActivationFunctionType.Exp

    with tile.TileContext(nc) as tc:
        with (
            tc.tile_pool(name="state", bufs=1) as st,
            tc.tile_pool(name="feat", bufs=3) as fp,
            tc.tile_pool(name="ef", bufs=3) as efp,
            tc.tile_pool(name="vps", bufs=2, space="PSUM") as vps,
            tc.tile_pool(name="bcps", bufs=1, space="PSUM") as bcps,
            tc.tile_pool(name="zps", bufs=2, space="PSUM") as zps,
            tc.tile_pool(name="sps", bufs=1, space="PSUM") as sps,
            tc.tile_pool(name="misc", bufs=1) as mp,
        ):
            # ---- one-time setup ----
            tr_sb = st.tile([L, L], f32)
            nc.sync.dma_start(tr_sb[:], trans[:])
            nc0 = st.tile([L, 1], f32)          # bias tile: -C0
            nc.vector.memset(nc0[:], -C0)
            Ep = st.tile([L, L], f32)           # E' = exp(trans - C0)
            nc.scalar.activation(Ep[:], tr_sb[:], EXP, bias=nc0[:], scale=1.0)
            texp = st.tile([L, 1], f32)         # exp(trans[:, EOS])
            zb = st.tile([L, 1], f32)
            nc.vector.memset(zb[:], 0.0)
            nc.scalar.activation(texp[:], tr_sb[:, EOS:EOS + 1], EXP,
                                 bias=zb[:], scale=1.0)
            ebos = st.tile([L, 1], f32)         # exp(trans[BOS, :]) as a column
            nc.sync.dma_start(ebos[:], trans[BOS:BOS + 1, :].rearrange("a b -> b a"))
            nc.scalar.activation(ebos[:], ebos[:], EXP, bias=zb[:], scale=1.0)
            ones_col = st.tile([L, 1], f32)     # lhsT for column sums
            nc.vector.memset(ones_col[:], 1.0)
            ones_row = st.tile([1, L], f32)     # lhsT for partition broadcast
            nc.vector.memset(ones_row[:], 1.0)

            wring = st.tile([L, RING * BC], f32)
            cacc = st.tile([1, CH * BC], f32)
            nc.vector.memset(cacc[:], 0.0)
            recips = st.tile([1, NEV * BC], f32)

            # ---- init: w_0 = exp(trans[BOS, :])[:,None] * exp(feat_0) ----
            ft0 = fp.tile([L, CH * BC], f32, tag="ftile")
            nc.sync.dma_start(ft0[:], featc[0])
            ef = efp.tile([L, CH * BC], f32, tag="ef")
            nc.scalar.activation(ef[:], ft0[:], EXP, bias=zb[:], scale=1.0)
            nc.vector.tensor_scalar(out=wring[:, 0:BC], in0=ef[:, 0:BC],
                                    scalar1=ebos[:, 0:1], scalar2=None, op0=MUL)

            # ---- recurrence over t = 1..T-1 ----
            for t in range(1, T):
                c, j = t // CH, t % CH
                s, sp = (t % RING) * BC, ((t - 1) % RING) * BC
                if j == 0:  # new feature chunk
                    ft = fp.tile([L, CH * BC], f32, tag="ftile")
                    nc.sync.dma_start(ft[:], featc[c])
                    ef = efp.tile([L, CH * BC], f32, tag="ef")
                    nc.scalar.activation(ef[:], ft[:], EXP, bias=zb[:], scale=1.0)

                v = vps.tile([L, BC], f32, space="PSUM")
                nc.tensor.matmul(v[:], lhsT=Ep[:], rhs=wring[:, sp:sp + BC],
                                 start=True, stop=True)
                nc.vector.tensor_tensor(out=wring[:, s:s + BC], in0=v[:],
                                        in1=ef[:, j * BC:(j + 1) * BC], op=MUL)

                if j == CH - 1:  # capture chunk c: slots half*8 .. half*8+7
                    half = ((t % RING) // CH) * CH * BC
                    z = zps.tile([1, CH * BC], f32, space="PSUM")
                    nc.tensor.matmul(z[:], lhsT=texp[:],
                                     rhs=wring[:, half:half + CH * BC],
                                     start=True, stop=True)
                    ind_row = efp.tile([1, CH * BC], f32, tag="indrow")
                    nc.sync.dma_start(ind_row[:], ind[c:c + 1, :])
                    zi = mp.tile([1, CH * BC], f32, tag="zi")
                    nc.vector.tensor_tensor(out=zi[:], in0=z[:],
                                            in1=ind_row[:], op=MUL)
                    nc.vector.tensor_tensor(out=cacc[:], in0=cacc[:], in1=zi[:],
                                            op=ADD)

                if t % RING == RING - 1 and t != T - 1:  # rescale event
                    ev = (t - (RING - 1)) // RING
                    cs = sps.tile([1, BC], f32, space="PSUM")
                    nc.tensor.matmul(cs[:], lhsT=ones_col[:],
                                     rhs=wring[:, s:s + BC], start=True, stop=True)
                    rc = recips[:, ev * BC:(ev + 1) * BC]
                    nc.vector.reciprocal(rc, cs[:])
                    bc_ps = bcps.tile([L, BC], f32, space="PSUM")
                    nc.tensor.matmul(bc_ps[:], lhsT=ones_row[:],
                                     rhs=rc, start=True, stop=True)
                    nc.vector.tensor_tensor(out=wring[:, s:s + BC],
                                            in0=wring[:, s:s + BC], in1=bc_ps[:],
                                            op=MUL)

            # ---- gold score masked sums ----
            ev_sb = mp.tile([BC, T], f32, tag="gv")
            nc.sync.dma_start(ev_sb[:], emis_v[:])
            ew_sb = mp.tile([BC, T], f32, tag="gw")
            nc.sync.dma_start(ew_sb[:], emis_w[:])
            nc.vector.tensor_tensor(out=ev_sb[:], in0=ev_sb[:], in1=ew_sb[:], op=MUL)
            g1 = mp.tile([BC, 1], f32, tag="g1")
            nc.vector.reduce_sum(g1[:], ev_sb[:], axis=AX)

            tv_sb = mp.tile([BC, T + 1], f32, tag="tv")
            nc.sync.dma_start(tv_sb[:], trans_v[:])
            tw_sb = mp.tile([BC, T + 1], f32, tag="tw")
            nc.sync.dma_start(tw_sb[:], trans_w[:])
            nc.vector.tensor_tensor(out=tv_sb[:], in0=tv_sb[:], in1=tw_sb[:], op=MUL)
            g2 = mp.tile([BC, 1], f32, tag="g2")
            nc.vector.reduce_sum(g2[:], tv_sb[:], axis=AX)
            nc.vector.tensor_tensor(out=g1[:], in0=g1[:], in1=g2[:], op=ADD)

            # ---- outputs ----
            nc.sync.dma_start(gold_o[:], g1[:])
            nc.sync.dma_start(cacc_o[:], cacc[:])
            nc.sync.dma_start(recips_o[:], recips[:])

    nc.compile()
    return nc


def _get_compiled():
    global _compiled
    if _compiled is None:
        _compiled = _build()
    return _compiled


def _prep_core(feat, tags, maskf, trans_np):
    """Host-side marshalling for one core's shard (no float arithmetic)."""
    # feature chunks: featc[c, l, ch*BC + b] = feat[b, 8c+ch, l]
    fc = feat.transpose(1, 2, 0)                       # [T, L, BC]
    fc = fc.reshape(NCHUNK, CH, L, BC).transpose(0, 2, 1, 3)  # [NCHUNK, L, CH, BC]
    featc = np.ascontiguousarray(fc.reshape(NCHUNK, L, CH * BC))

    lens = maskf.sum(axis=1).astype(np.int64)          # in [T//2, T]
    tstar = lens - 1                                   # capture step per seq

    ind = np.zeros((NCHUNK, CH * BC), dtype=F32)
    k = tstar // CH
    tpp = tstar % CH
    ind[k, tpp * BC + np.arange(BC)] = 1.0

    emis_v = np.take_along_axis(feat, tags[..., None], axis=-1)[..., 0]  # [BC,T]
    emis_w = maskf.copy()
    emis_w[:, 0] = 1.0

    trans_v = np.empty((BC, T + 1), dtype=F32)
    trans_v[:, : T - 1] = trans_np[tags[:, :-1], tags[:, 1:]]
    trans_v[:, T - 1] = trans_np[BOS, tags[:, 0]]
    last_lab = tags[np.arange(BC), tstar]
    trans_v[:, T] = trans_np[last_lab, EOS]
    trans_w = np.empty((BC, T + 1), dtype=F32)
    trans_w[:, : T - 1] = maskf[:, 1:]
    trans_w[:, T - 1] = 1.0
    trans_w[:, T] = 1.0

    in_map = {
        "featc": featc,
        "trans": np.ascontiguousarray(trans_np),
        "ind": ind,
        "emis_v": np.ascontiguousarray(emis_v.astype(F32)),
        "emis_w": np.ascontiguousarray(emis_w),
        "trans_v": trans_v,
        "trans_w": trans_w,
    }
    return in_map, tstar


def _prep_all(inputs):
    feats = np.asarray(inputs["features"], dtype=F32)
    tags = np.asarray(inputs["tag_seqs"])
    maskf = np.asarray(inputs["mask"]).astype(F32)
    trans_np = np.asarray(inputs["transitions"], dtype=F32)
    in_maps = []
    for c in range(NCORES):
        sl = slice(c * BC, (c + 1) * BC)
        m, _ = _prep_core(feats[sl], tags[sl], maskf[sl], trans_np)
        in_maps.append(m)
    return in_maps


def kernel(features, tag_seqs, mask, transitions):
    from concourse import bass_utils

    feats = np.asarray(features, dtype=F32)
    tags = np.asarray(tag_seqs)
    maskf = np.asarray(mask).astype(F32)
    trans_np = np.asarray(transitions, dtype=F32)

    nc = _get_compiled()

    in_maps, tstars = [], []
    for c in range(NCORES):
        sl = slice(c * BC, (c + 1) * BC)
        m, ts = _prep_core(feats[sl], tags[sl], maskf[sl], trans_np)
        in_maps.append(m)
        tstars.append(ts)

    res = bass_utils.run_bass_kernel_spmd(nc, in_maps, core_ids=list(range(NCORES)))

    ev_t = (RING - 1) + RING * np.arange(NEV)          # rescale step of event ev
    per_seq = []
    for c in range(NCORES):
        out = res.results[c]
        ts = tstars[c]
        Cb = out["cacc"].reshape(CH, BC).sum(axis=0)   # captured z_{t*}[b]
        logs = -np.log(out["recips"].reshape(NEV, BC))  # [NEV, BC] log colsums
        applies = ev_t[:, None] < ts[None, :]          # event strictly before t*
        logZ = np.log(Cb) + ts * C0 + (logs * applies).sum(axis=0)
        gold = out["gold"][:, 0]
        per_seq.append(gold - logZ)

    loss = -np.mean(np.concatenate(per_seq))
    return np.float32(loss)



# revision 4
# speedup vs baseline: 2.4581x; 2.4581x over previous
"""CRF NLL loss kernel for Trainium2 (8 NeuronCores, batch-sharded).

Strategy (v2)
-------------
Data-parallel over batch: each of 8 cores handles BC=64 sequences.

Forward algorithm in the EXP DOMAIN with labels on partitions, batch on
the free dim: w_t[l, b] ~ exp(fv_t[l, b] - t*C0).  One step is a single
bf16 PE matmul with stationary Ep2 = exp(transitions - C0) plus one DVE
multiply by exp(features_t):

    w_t = ef_t * (Ep2^T @ w_{t-1})

Capture trick: labels PAD(0) and BOS(1) have identically-zero forward
mass under the CRF's constrained transitions, so column 0 of Ep2 is
replaced by texp = exp(trans[:, EOS]) (with texp[PAD/BOS] := 0) and row
0 of Ep2 is zeroed.  Then row 0 of every matmul output carries
z_{t-1} = sum_p exp(trans[p,EOS]) * w_{t-1}[p] -- the log-partition
numerator -- for free.  Host feature marshalling zeroes feature row 0
(so ef[0] = 1) and w_t[0] = z_{t-1} rides along in the state; every 16
steps the ring row 0 is DMA'd out, and the host selects z at t* = len-1
per sequence.

Rescaling: every 16 steps rc = 1/w[0] (the z row, bf16) is recorded and
applied OFF the critical path to a future emission tile (17 steps
later), broadcast across partitions with a 1-partition matmul.  The
host un-does the logged rc factors in log space (events with
s_app <= t*+1; the export step carries the factor applied at it).

Gold path score: host gathers the indexed scalars feat[b,t,tag] and
trans[tag,tag']; the device does the masked weighted sums.

All matmuls are bf16 (one PE pass instead of fp32's two); bf16 keeps
fp32's exponent range so the exp-domain state cannot over/underflow any
faster, and the loss tolerance (2e-2 relative on a ~1e5 loss) dwarfs
bf16 rounding.
"""

import numpy as np

B, T, L = 512, 512, 128
NCORES = 8
BC = B // NCORES            # 64 sequences per core
PAD, BOS, EOS = 0, 1, 2
C0 = 5.0                    # constant per-step log-shift folded into Ep2
CH = 8                      # steps per feature chunk
NCHUNK = T // CH            # 64 chunks
RING = 32                   # w ring slots
NEV = 30                    # rescale events: measured at t=15+16ev, applied at t=32+16ev
PREF = 2                    # chunks prefetched ahead

F32 = np.float32

_compiled = None


def _build():
    import concourse.bass as bass
    import concourse.bacc as bacc
    import concourse.mybir as mybir
    import concourse.tile as tile

    f32 = mybir.dt.float32
    bf16 = mybir.dt.bfloat16
    nc = bacc.Bacc("TRN2", target_bir_lowering=False, debug=False)

    featc = nc.dram_tensor("featc", [NCHUNK, L, CH * BC], f32, kind="ExternalInput")
    ep2 = nc.dram_tensor("ep2", [L, L], bf16, kind="ExternalInput")
    emis_v = nc.dram_tensor("emis_v", [BC, T], f32, kind="ExternalInput")
    emis_w = nc.dram_tensor("emis_w", [BC, T], f32, kind="ExternalInput")
    trans_v = nc.dram_tensor("trans_v", [BC, T + 1], f32, kind="ExternalInput")
    trans_w = nc.dram_tensor("trans_w", [BC, T + 1], f32, kind="ExternalInput")

    zrows_o = nc.dram_tensor("zrows", [T // 16, 16 * BC], bf16, kind="ExternalOutput")
    zlast_o = nc.dram_tensor("zlast", [1, BC], f32, kind="ExternalOutput")
    recips_o = nc.dram_tensor("recips", [1, NEV * BC], bf16, kind="ExternalOutput")
    gold_o = nc.dram_tensor("gold", [BC, 1], f32, kind="ExternalOutput")

    AX = mybir.AxisListType.X
    MUL = mybir.AluOpType.mult
    ADD = mybir.AluOpType.add
    EXP = mybir.ActivationFunctionType.Exp

    with tile.TileContext(nc) as tc:
        with (
            tc.tile_pool(name="state", bufs=1) as st,
            tc.tile_pool(name="feat", bufs=PREF + 1) as fp,
            tc.tile_pool(name="ef", bufs=PREF + 1) as efp,
            tc.tile_pool(name="vps", bufs=4, space="PSUM") as vps,
            tc.tile_pool(name="bcps", bufs=2, space="PSUM") as bcps,
            tc.tile_pool(name="zps", bufs=1, space="PSUM") as zps,
            tc.tile_pool(name="misc", bufs=1) as mp,
        ):
            # ---- one-time setup ----
            ep2_sb = st.tile([L, L], bf16)
            nc.sync.dma_start(ep2_sb[:], ep2[:])
            ones_row = st.tile([1, L], bf16)    # lhsT for partition broadcast
            nc.vector.memset(ones_row[:], 1.0)

            wring = st.tile([L, RING * BC], bf16)
            recips = st.tile([1, NEV * BC], bf16)

            # ---- chunk prep helper ----
            ef_tiles = {}

            def prep_chunk(c):
                if c >= NCHUNK:
                    return
                ft = fp.tile([L, CH * BC], f32, tag="ftile")
                nc.sync.dma_start(ft[:], featc[c])
                ef = efp.tile([L, CH * BC], bf16, tag="ef")
                nc.scalar.activation(ef[:], ft[:], EXP, bias=0.0, scale=1.0)
                ef_tiles[c] = ef

            for c in range(PREF + 1):
                prep_chunk(c)

            # ---- init: w_0 = ef_0[:, 0:BC] (BOS row folded into feat t=0) ----
            nc.vector.tensor_copy(wring[:, 0:BC], ef_tiles[0][:, 0:BC])

            # ---- recurrence over t = 1..T-1 ----
            for t in range(1, T):
                c, j = t // CH, t % CH
                s, sp = (t % RING) * BC, ((t - 1) % RING) * BC
                if j == 0:
                    prep_chunk(c + PREF)
                    del ef_tiles[c - 1]
                    # rescale application onto this chunk's first block
                    if c >= 4 and c % 2 == 0 and (c - 4) // 2 < NEV:
                        ev = (c - 4) // 2
                        bc_ps = bcps.tile([L, BC], f32, space="PSUM")
                        nc.tensor.matmul(bc_ps[:], lhsT=ones_row[:],
                                         rhs=recips[:, ev * BC:(ev + 1) * BC],
                                         start=True, stop=True)
                        efc = ef_tiles[c]
                        nc.vector.tensor_tensor(out=efc[:, 0:BC], in0=bc_ps[:],
                                                in1=efc[:, 0:BC], op=MUL)

                v = vps.tile([L, BC], f32, space="PSUM")
                nc.tensor.matmul(v[:], lhsT=ep2_sb[:], rhs=wring[:, sp:sp + BC],
                                 start=True, stop=True)
                nc.vector.tensor_tensor(out=wring[:, s:s + BC], in0=v[:],
                                        in1=ef_tiles[c][:, j * BC:(j + 1) * BC],
                                        op=MUL)

                if t % 16 == 15:
                    # record rescale reciprocal from the z row
                    ev = (t - 15) // 16
                    if ev < NEV:
                        with nc.allow_low_precision(
                            reason="bf16 rescale factor is logged and un-done "
                                   "exactly on the host"):
                            nc.vector.reciprocal(recips[:, ev * BC:(ev + 1) * BC],
                                                 wring[0:1, s:s + BC])
                    # export z rows (16 slots ending at slot of t)
                    w = (t - 15) // 16
                    lo = ((t - 15) % RING) * BC
                    nc.sync.dma_start(zrows_o[w:w + 1, :],
                                      wring[0:1, lo:lo + 16 * BC])

            # ---- final z_{T-1}: one more (1-col) matmul ----
            vz = zps.tile([1, BC], f32, space="PSUM")
            sl = ((T - 1) % RING) * BC
            nc.tensor.matmul(vz[:], lhsT=ep2_sb[:, 0:1], rhs=wring[:, sl:sl + BC],
                             start=True, stop=True)
            zl = mp.tile([1, BC], f32, tag="zl")
            nc.vector.tensor_copy(zl[:], vz[:])
            nc.sync.dma_start(zlast_o[:], zl[:])
            nc.sync.dma_start(recips_o[:], recips[:])

            # ---- gold score masked sums ----
            ev_sb = mp.tile([BC, T], f32, tag="gv")
            nc.sync.dma_start(ev_sb[:], emis_v[:])
            ew_sb = mp.tile([BC, T], f32, tag="gw")
            nc.sync.dma_start(ew_sb[:], emis_w[:])
            nc.vector.tensor_tensor(out=ev_sb[:], in0=ev_sb[:], in1=ew_sb[:], op=MUL)
            g1 = mp.tile([BC, 1], f32, tag="g1")
            nc.vector.reduce_sum(g1[:], ev_sb[:], axis=AX)

            tv_sb = mp.tile([BC, T + 1], f32, tag="tv")
            nc.sync.dma_start(tv_sb[:], trans_v[:])
            tw_sb = mp.tile([BC, T + 1], f32, tag="tw")
            nc.sync.dma_start(tw_sb[:], trans_w[:])
            nc.vector.tensor_tensor(out=tv_sb[:], in0=tv_sb[:], in1=tw_sb[:], op=MUL)
            g2 = mp.tile([BC, 1], f32, tag="g2")
            nc.vector.reduce_sum(g2[:], tv_sb[:], axis=AX)
            nc.vector.tensor_tensor(out=g1[:], in0=g1[:], in1=g2[:], op=ADD)
            nc.sync.dma_start(gold_o[:], g1[:])

    nc.compile()
    return nc


def _get_compiled():
    global _compiled
    if _compiled is None:
        _compiled = _build()
    return _compiled


def _host_consts(trans_np):
    import ml_dtypes

    Ep = np.exp(trans_np.astype(np.float64) - C0)
    texp = np.exp(trans_np[:, EOS].astype(np.float64))
    texp[PAD] = 0.0
    texp[BOS] = 0.0
    Ep2 = Ep.copy()
    Ep2[:, PAD] = texp            # output col 0 carries z
    Ep2[PAD, :] = 0.0             # z-row garbage leaks nowhere
    return np.ascontiguousarray(Ep2.astype(ml_dtypes.bfloat16))


def _prep_core(feat, tags, maskf, trans_np, ep2_bf16):
    """Host-side marshalling for one core's shard."""
    featm = feat.copy()
    featm[:, 0, :] += trans_np[BOS, :][None, :]
    featm[:, :, PAD] = 0.0        # ef row 0 == 1 -> w[0] = z passthrough
    fc = featm.transpose(1, 2, 0)                             # [T, L, BC]
    fc = fc.reshape(NCHUNK, CH, L, BC).transpose(0, 2, 1, 3)  # [NCHUNK,L,CH,BC]
    featc = np.ascontiguousarray(fc.reshape(NCHUNK, L, CH * BC))

    lens = maskf.sum(axis=1).astype(np.int64)
    tstar = lens - 1

    emis_v = np.take_along_axis(feat, tags[..., None], axis=-1)[..., 0]  # [BC,T]
    emis_w = maskf.copy()
    emis_w[:, 0] = 1.0

    trans_v = np.empty((BC, T + 1), dtype=F32)
    trans_v[:, : T - 1] = trans_np[tags[:, :-1], tags[:, 1:]]
    trans_v[:, T - 1] = trans_np[BOS, tags[:, 0]]
    last_lab = tags[np.arange(BC), tstar]
    trans_v[:, T] = trans_np[last_lab, EOS]
    trans_w = np.empty((BC, T + 1), dtype=F32)
    trans_w[:, : T - 1] = maskf[:, 1:]
    trans_w[:, T - 1] = 1.0
    trans_w[:, T] = 1.0

    in_map = {
        "featc": featc,
        "ep2": ep2_bf16,
        "emis_v": np.ascontiguousarray(emis_v.astype(F32)),
        "emis_w": np.ascontiguousarray(emis_w),
        "trans_v": trans_v,
        "trans_w": trans_w,
    }
    return in_map, tstar


def _prep_all(inputs):
    feats = np.asarray(inputs["features"], dtype=F32)
    tags = np.asarray(inputs["tag_seqs"])
    maskf = np.asarray(inputs["mask"]).astype(F32)
    trans_np = np.asarray(inputs["transitions"], dtype=F32)
    ep2_bf16 = _host_consts(trans_np)
    in_maps = []
    for c in range(NCORES):
        sl = slice(c * BC, (c + 1) * BC)
        m, _ = _prep_core(feats[sl], tags[sl], maskf[sl], trans_np, ep2_bf16)
        in_maps.append(m)
    return in_maps


def kernel(features, tag_seqs, mask, transitions):
    from concourse import bass_utils

    feats = np.asarray(features, dtype=F32)
    tags = np.asarray(tag_seqs)
    maskf = np.asarray(mask).astype(F32)
    trans_np = np.asarray(transitions, dtype=F32)

    nc = _get_compiled()
    ep2_bf16 = _host_consts(trans_np)

    in_maps, tstars = [], []
    for c in range(NCORES):
        sl = slice(c * BC, (c + 1) * BC)
        m, ts = _prep_core(feats[sl], tags[sl], maskf[sl], trans_np, ep2_bf16)
        in_maps.append(m)
        tstars.append(ts)

    res = bass_utils.run_bass_kernel_spmd(nc, in_maps, core_ids=list(range(NCORES)))

    s_app = 32 + 16 * np.arange(NEV)               # event ev applied at step s_app
    per_seq = []
    for c in range(NCORES):
        out = res.results[c]
        ts = tstars[c]                              # [BC]
        zr = np.asarray(out["zrows"]).astype(np.float64).reshape(T // 16, 16, BC)
        zlast = np.asarray(out["zlast"]).astype(np.float64)[0]
        rc = np.asarray(out["recips"]).astype(np.float64).reshape(NEV, BC)
        te = ts + 1                                 # export step of z_{t*}
        bidx = np.arange(BC)
        z_sel = np.where(te >= T, zlast, zr[np.minimum(te // 16, T // 16 - 1),
                                           te % 16, bidx])
        applies = s_app[:, None] <= te[None, :]     # export step carries its factor
        logcorr = (-np.log(rc) * applies).sum(axis=0)
        logZ = np.log(z_sel) + ts * C0 + logcorr
        gold = np.asarray(out["gold"]).astype(np.float64)[:, 0]
        per_seq.append(gold - logZ)

    loss = -np.mean(np.concatenate(per_seq))
    return np.float32(loss)
